# revision 2
# baseline (speedup 1.0000x reference)
"""CASSI colored-aperture layer v2.1: PE-offloaded H construction.

Per core (128 partitions = m-rows, free dims s-major (s, n)):
  setup (chunk-pipelined): W = sum_c w_c, r = 1/W, a_c = w_c*r in place.
  stage 1 per band l:
    h_l = sum_{c<3} (F[c,l]-F[3,l]) * a_c + F[3,l]
      PE: 3 accumulating matmuls per 512-col slice, stationary weights =
      diag(F[c,l]-F[3,l]) (host-built, DMA-streamed per band); f32 PSUM
      chunks ping-pong; ACT evacuates PSUM -> SBUF f16 adding F[3,l].
      Per chunk: p = h*x_l (DVE, s-broadcast), Y[:, s, l+n] += p
      (Pool/DVE per-chunk knob).  h_l spilled to DRAM except KEEP trailing
      bands (SBUF) and RECOMP leading bands (recomputed on PE in stage 2).
  stage 2 per band l (kept bands, then recomputed, then DMA-prefetched):
    t = h_l * Y[:, :, l:l+N]  (DVE high stripes / Pool low stripes)
    shot-sum: PE-tree bands use 11 matmuls, each folding a PAIR of
    s-stripes (512 moving cols, stride-0 out AP) into an f32 X_l PSUM
    accumulator (ACT evac, DMA out); other bands use an all-DVE halving
    tree. Pipeline fp16 with f32 PSUM accumulation.
"""

import numpy as np

B, M, N, L, S = 4, 256, 256, 24, 22
MSH = M // 2                     # rows per core
NCORES = 8
NS, NL = N * S, N * L
NP = N + L - 1                   # 279 shifted columns
YW = NP * S                      # Y free width (s-major)

KEEP = 3                         # trailing bands kept in SBUF (no spill)
RECOMP = 3                       # leading bands recomputed on PE in stage 2
CHUNKS = [(0, 1536), (1536, 1536), (3072, 1536), (4608, 1024)]  # psum chunks
POOL_Y_CHUNKS = (0, 3)           # Yacc chunk indices on Pool (rest DVE)
DVE_T_STRIPES = 18               # stage2 t-mult: stripes on DVE (rest Pool)
# stage2 bands whose shot-sum runs on PE (rest: all-DVE halving tree)
PE_TREE_LS = frozenset(range(L))
# bands whose c=2 h-term is applied by DVE (TS+TT) instead of a PE stream
MOVE_LS = frozenset({1, 4, 7, 10, 13, 16, 19, 22})


def _bases() -> np.ndarray:
    wl = np.linspace(400.0, 700.0, L)

    def g(mu: float, sig: float) -> np.ndarray:
        return np.exp(-0.5 * ((wl - mu) / sig) ** 2)

    return np.stack([g(620.0, 50.0), g(550.0, 50.0), g(450.0, 50.0), g(500.0, 50.0)])


_F = _bases()
_C = (_F[:3] - _F[3]).astype(np.float64)     # 3 x L coefficients
_C3 = _F[3].astype(np.float64)               # bias per l


def _diag_blocks() -> np.ndarray:
    """[128, (L*3+1)*128] f16: diag(C[c,l]) blocks (l,c) row-major, identity last."""
    blocks = np.zeros((MSH, (L * 3 + 1) * MSH), dtype=np.float16)
    idx = np.arange(MSH)
    for l in range(L):
        for c in range(3):
            blocks[idx, (l * 3 + c) * MSH + idx] = np.float16(_C[c, l])
    blocks[idx, L * 3 * MSH + idx] = np.float16(1.0)
    return blocks


_NC = None


def _build():
    import concourse.bacc as bacc
    import concourse.mybir as mybir
    import concourse.tile as tile

    f16, f32 = mybir.dt.float16, mybir.dt.float32
    A = mybir.AluOpType
    Copy = mybir.ActivationFunctionType.Copy

    nc = bacc.Bacc("TRN2", target_bir_lowering=False, debug=False, num_devices=NCORES)
    xin = nc.declare_dram_parameter("x16", [MSH, NL], f16, isOutput=False)   # (l, n)
    wins = [
        nc.declare_dram_parameter(f"w{i}", [MSH, NS], f16, isOutput=False)   # (s, n)
        for i in range(4)
    ]
    dgin = nc.declare_dram_parameter("diags", [MSH, (L * 3 + 1) * MSH], f16,
                                     isOutput=False)
    out = nc.declare_dram_parameter("out", [MSH, NL], f32, isOutput=True)    # (l, n)
    hcache = nc.dram_tensor("hcache", [L, MSH, NS], f16)

    with tile.TileContext(nc) as tc:
        with (
            tc.tile_pool(name="main", bufs=1) as main,
            tc.tile_pool(name="hp", bufs=4) as hp,
            tc.tile_pool(name="pp", bufs=2) as pp,
            tc.tile_pool(name="dgp", bufs=4) as dgp,
            tc.tile_pool(name="ps", bufs=1, space="PSUM") as ps,
        ):
            w = [main.tile([MSH, NS], f16, tag=f"w{i}", name=f"w{i}t") for i in range(4)]
            xt = main.tile([MSH, NL], f16, tag="x", name="xt")
            Y = main.tile([MSH, YW], f16, tag="Y", name="Yt")
            ident = main.tile([MSH, MSH], f16, tag="id", name="identt")

            for i in range(4):
                nc.sync.dma_start(w[i][:], wins[i][:])
            nc.sync.dma_start(ident[:], dgin[:, L * 3 * MSH : (L * 3 + 1) * MSH])
            nc.sync.dma_start(xt[:], xin[:])
            nc.gpsimd.memset(Y[:], 0.0)

            # a_c = w_c / (w0+w1+w2+w3) in place, pipelined by 1408-col chunks
            u = hp.tile([MSH, NS], f16, tag="h", name="ut")
            for q0 in range(0, NS, 1408):
                q = slice(q0, q0 + 1408)
                nc.vector.tensor_tensor(u[:, q], w[0][:, q], w[1][:, q], A.add)
                nc.vector.tensor_tensor(u[:, q], u[:, q], w[2][:, q], A.add)
                nc.vector.tensor_tensor(u[:, q], u[:, q], w[3][:, q], A.add)
                with nc.allow_low_precision("fp16 pipeline, ~1e-3 validated"):
                    nc.vector.reciprocal(u[:, q], u[:, q])
                for i in range(3):
                    nc.vector.tensor_tensor(w[i][:, q], w[i][:, q], u[:, q], A.mult)
            a = w[:3]

            x3 = xt[:].rearrange("p (l n) -> p l n", n=N)
            Y3 = Y[:].rearrange("p (s n) -> p s n", n=NP)

            def build_h(l, h, dgl):
                """PE h-construction + ACT evac; yields per-chunk via callback."""
                ncs = 2 if l in MOVE_LS else 3
                for ci, (off, width) in enumerate(CHUNKS):
                    pt = ps.tile([MSH, 1536], f32, tag="hps", bufs=2,
                                 name=f"ps{l}_{ci}")
                    for j0 in range(0, width, 512):
                        for c in range(ncs):
                            nc.tensor.matmul(
                                pt[:, j0 : j0 + 512],
                                dgl[:, c * MSH : (c + 1) * MSH],
                                a[c][:, off + j0 : off + j0 + 512],
                                start=(c == 0),
                                stop=(c == ncs - 1),
                            )
                    nc.scalar.activation(
                        h[:, off : off + width], pt[:, :width], Copy,
                        bias=float(_C3[l]), scale=1.0,
                    )
                    if ncs == 2:
                        tm = pp.tile([MSH, 1536], f16, tag="tm", bufs=3,
                                     name=f"tm{l}_{ci}")
                        nc.vector.tensor_scalar_mul(
                            tm[:, :width], a[2][:, off : off + width], float(_C[2, l])
                        )
                        nc.vector.tensor_tensor(
                            h[:, off : off + width], h[:, off : off + width],
                            tm[:, :width], A.add,
                        )
                    yield ci, off, width

            def load_dg(l):
                dgl = dgp.tile([MSH, 3 * MSH], f16, tag="dg", name=f"dg{l}")
                nc.sync.dma_start(dgl[:], dgin[:, l * 3 * MSH : (l + 1) * 3 * MSH])
                return dgl

            # ---------------- stage 1 ----------------
            kept = {}
            for l in range(L):
                dgl = load_dg(l)
                if l >= L - KEEP:
                    h = main.tile([MSH, NS], f16, tag=f"hk{l}", name=f"hk{l}")
                else:
                    h = hp.tile([MSH, NS], f16, tag="h", name=f"h{l}")

                p = pp.tile([MSH, NS], f16, tag="p", bufs=3, name=f"p{l}")
                for ci, off, width in build_h(l, h, dgl):
                    # p = h * x_l on this chunk (broadcast over its s-range)
                    s0, s1 = off // N, (off + width) // N
                    xb = x3[:, l, :].unsqueeze(1).broadcast_to((MSH, s1 - s0, N))
                    nc.vector.tensor_tensor(
                        p[:, off : off + width].rearrange("p (s n) -> p s n", n=N),
                        h[:, off : off + width].rearrange("p (s n) -> p s n", n=N),
                        xb,
                        A.mult,
                    )
                    ysl = Y3[:, s0:s1, l : l + N]
                    pv = p[:, off : off + width].rearrange("p (s n) -> p s n", n=N)
                    if ci in POOL_Y_CHUNKS:
                        nc.gpsimd.tensor_tensor(ysl, ysl, pv, A.add)
                    else:
                        nc.vector.tensor_tensor(ysl, ysl, pv, A.add)

                if l >= L - KEEP:
                    kept[l] = h
                elif l >= RECOMP:
                    nc.sync.dma_start(hcache[l], h[:])

            # ---------------- stage 2 ----------------
            order = (list(range(L - KEEP, L)) + list(range(RECOMP))
                     + list(range(RECOMP, L - KEEP)))
            for l in order:
                if l in kept:
                    h = kept[l]
                elif l < RECOMP:
                    h = hp.tile([MSH, NS], f16, tag="h", name=f"hr{l}")
                    dgl = load_dg(l)
                    for _ci, _off, _w in build_h(l, h, dgl):
                        pass
                else:
                    # recycle stage-1 tile slots (w0..w3, x) for deep prefetch
                    ridx = l % 5
                    if ridx < 4:
                        h = main.tile([MSH, NS], f16, tag=f"w{ridx}", name=f"hr{l}")
                    else:
                        h = hp.tile([MSH, NS], f16, tag="h", name=f"hr{l}")
                    nc.sync.dma_start(h[:], hcache[l])
                t = pp.tile([MSH, NS], f16, tag="p", bufs=3, name=f"t{l}")
                t3 = t[:].rearrange("p (s n) -> p s n", n=N)
                h3 = h[:].rearrange("p (s n) -> p s n", n=N)
                d = DVE_T_STRIPES
                nc.vector.tensor_tensor(
                    t3[:, :d, :], h3[:, :d, :], Y3[:, :d, l : l + N], A.mult
                )
                if d < S:
                    nc.gpsimd.tensor_tensor(
                        t3[:, d:, :], h3[:, d:, :], Y3[:, d:, l : l + N], A.mult
                    )
                if l in PE_TREE_LS:
                    # 11 paired-stripe matmuls into an f32 PSUM accumulator
                    xp = ps.tile([MSH, 256], f32, tag="xps", bufs=2,
                                 name=f"xp{l}")
                    xpb = xp[:].unsqueeze(1).broadcast_to((MSH, 2, 256))
                    for k in range(11):
                        nc.tensor.matmul(
                            xpb, ident[:], t3[:, 2 * k : 2 * k + 2, :],
                            start=(k == 0), stop=(k == 10),
                        )
                    xo = pp.tile([MSH, 256], f32, tag="xo", bufs=2, name=f"xo{l}")
                    nc.scalar.activation(xo[:], xp[:], Copy)
                    nc.scalar.dma_start(out[:, l * N : (l + 1) * N], xo[:])
                else:
                    # all-DVE halving tree (f16, final add to f32)
                    tv = t[:]
                    nc.vector.tensor_tensor(
                        tv[:, : 11 * N], tv[:, : 11 * N], tv[:, 11 * N : 22 * N], A.add
                    )
                    nc.vector.tensor_tensor(
                        tv[:, : 5 * N], tv[:, : 5 * N], tv[:, 5 * N : 10 * N], A.add
                    )
                    nc.vector.tensor_tensor(
                        tv[:, : 2 * N], tv[:, : 2 * N], tv[:, 2 * N : 4 * N], A.add
                    )
                    nc.vector.tensor_tensor(
                        tv[:, :N], tv[:, :N], tv[:, N : 2 * N], A.add
                    )
                    nc.vector.tensor_tensor(
                        tv[:, :N], tv[:, :N], tv[:, 4 * N : 5 * N], A.add
                    )
                    xol = pp.tile([MSH, N], f32, tag="xo", bufs=2, name=f"xol{l}")
                    nc.vector.tensor_tensor(
                        xol[:], tv[:, :N], tv[:, 10 * N : 11 * N], A.add
                    )
                    nc.sync.dma_start(out[:, l * N : (l + 1) * N], xol[:])

    nc.compile()
    return nc


def _get_nc():
    global _NC
    if _NC is None:
        _NC = _build()
    return _NC


_DIAGS = None


def _make_in_maps(x, wr, wg, wb, wc):
    global _DIAGS
    if _DIAGS is None:
        _DIAGS = _diag_blocks()
    x = np.asarray(x, dtype=np.float32)
    ws = [np.asarray(w, dtype=np.float32).reshape(M, M, S) for w in (wr, wg, wb, wc)]
    in_maps = []
    for core in range(NCORES):
        b, mh = divmod(core, 2)
        rows = slice(mh * MSH, (mh + 1) * MSH)
        xs = x[b, rows].transpose(0, 2, 1)            # (MSH, L, N)
        m = {
            "x16": np.ascontiguousarray(xs).reshape(MSH, NL).astype(np.float16),
            "diags": _DIAGS,
        }
        for i, w in enumerate(ws):
            wsb = w[rows].transpose(0, 2, 1)          # (MSH, S, N)
            m[f"w{i}"] = np.ascontiguousarray(wsb).reshape(MSH, NS).astype(np.float16)
        in_maps.append(m)
    return in_maps


def _run_shards(in_maps):
    from concourse.bass_utils import run_bass_kernel_spmd

    nc = _get_nc()
    return run_bass_kernel_spmd(nc, in_maps, list(range(NCORES)))


def kernel(x, wr, wg, wb, wc):
    res = _run_shards(_make_in_maps(x, wr, wg, wb, wc))
    X = np.empty((B, M, N, L), dtype=np.float32)
    for core in range(NCORES):
        b, mh = divmod(core, 2)
        xo = res.results[core]["out"].reshape(MSH, L, N).transpose(0, 2, 1)
        X[b, mh * MSH : (mh + 1) * MSH] = xo
    return X / X.max()


def estimate_ns() -> float:
    from concourse.timeline_sim import TimelineSim

    return TimelineSim(_get_nc()).simulate()
